# revision 5
# baseline (speedup 1.0000x reference)
"""Fused MHA-layer Bass kernel for TRN2, SPMD over 8 NeuronCores.

Reference computation (per batch b):
    q = x@wq + bq ; k = x@wk + bk ; v = x@wv + bv          (full 1024-dim, no head split)
    s = (q @ k^T) / 8 ; s[mask] = 1e-9 ; attn = softmax(s)
    ctx = attn @ v + x
    out = layernorm(ctx @ wo + bo) * gamma + beta
Returns (out [B,S,D], attn [B,S,S]).

Sharding: 8 cores = 4 batches x 2 query-row blocks of 1024 rows. Each core
recomputes K/V for its whole batch (cheaper than cross-core exchange), and
computes its 1024 query rows end to end.

On-core layout notes:
  - Matmul computes lhsT.T @ rhs with contraction along partitions, so every
    left operand lives transposed: q^T/k^T are built directly as [d, row]
    via PE-transposed x chunks; attn is PE-transposed per 128x128 tile for
    the context matmul; ctx is PE-transposed for the output projection.
  - The 1/sqrt(d_k)=0.125 scale is folded into the ACT exp (exact: power of
    two), with the masked fill done in the raw-score domain as 8e-9 so the
    scaled value is bitwise fp32(1e-9), matching the reference.
  - Everything stays fp32. SBUF can't hold k^T, q^T and V (f32) at once, so
    the kernel runs in phases: scores/softmax write the normalized attn to
    DRAM while k^T/q^T are resident; then k^T/q^T are freed, V is built, and
    the attn rows are streamed back in for the context matmul.
"""

import numpy as np
from contextlib import ExitStack

import concourse.bass as bass
import concourse.mybir as mybir
import concourse.tile as tile
from concourse.masks import make_identity

F32 = mybir.dt.float32
BF16 = mybir.dt.bfloat16
U8 = mybir.dt.uint8
AX = mybir.AxisListType.X
EXP = mybir.ActivationFunctionType.Exp
SQRT = mybir.ActivationFunctionType.Sqrt
ADD = mybir.AluOpType.add
SUB = mybir.AluOpType.subtract
MULT = mybir.AluOpType.mult

MASK_FILL_RAW = float(np.float32(1e-9) * np.float32(8.0))  # /8 -> fp32(1e-9) exactly
LN_EPS = 1e-5


def _split(total, size):
    return [(i, min(size, total - i)) for i in range(0, total, size)]


def split_waits(nc, max_waits=1):
    """Walrus codegen in this toolchain accepts at most one sem-wait per
    instruction; Tile can attach several (e.g. on the exit drain). Hoist the
    extras onto preceding NOPs on the same engine (engine queues are FIFO, so
    semantics are unchanged)."""
    n_split = 0
    for f in nc.m.functions:
        for b in f.blocks:
            out, changed = [], False
            for inst in list(b.instructions):
                si = inst.sync_info
                if si is not None and si.on_wait is not None and len(si.on_wait) > max_waits:
                    waits = list(si.on_wait)
                    extra, keep = waits[:-max_waits], waits[-max_waits:]
                    for i, w in enumerate(extra):
                        out.append(mybir.InstNoOp(
                            name=f"{inst.name}-wsplit{i}",
                            engine=inst.engine, ins=[], outs=[],
                            sync_info=mybir.SyncInfo(on_wait=[w], on_update=[]),
                        ))
                        n_split += 1
                    inst.sync_info = mybir.SyncInfo(on_wait=keep, on_update=si.on_update)
                    changed = True
                out.append(inst)
            if changed:
                b.instructions[:] = out
    return n_split


def _bcast(ap, parts):
    """Partition-broadcast DMA source AP for a 1-D DRAM tensor."""
    return bass.AP(tensor=ap.tensor, offset=ap.offset, ap=[[0, parts]] + list(ap.ap))


def build_mha(S=2048, D=1024, BLK=1024, CH=256):
    """Build the per-core Bass program. One core handles BLK query rows of one
    batch; xf is the full [S, D] batch slice (for K/V), xq its [BLK, D] query
    rows, mask the [BLK, S] attention mask as uint8."""
    KD = D // 128   # d-tiles
    NT = BLK // 128  # query-row tiles
    NKC = S // 128  # key-row tiles

    nc = bass.Bass("TRN2")
    xf_h = nc.dram_tensor("xf", [S, D], F32, kind="ExternalInput")[:]
    xq_h = nc.dram_tensor("xq", [BLK, D], F32, kind="ExternalInput")[:]
    mask_h = nc.dram_tensor("mask", [BLK, S], U8, kind="ExternalInput")[:]
    wq_h = nc.dram_tensor("wq", [D, D], F32, kind="ExternalInput")[:]
    wk_h = nc.dram_tensor("wk", [D, D], F32, kind="ExternalInput")[:]
    wv_h = nc.dram_tensor("wv", [D, D], F32, kind="ExternalInput")[:]
    wo_h = nc.dram_tensor("wo", [D, D], F32, kind="ExternalInput")[:]
    bq_h = nc.dram_tensor("bq", [D], F32, kind="ExternalInput")[:]
    bk_h = nc.dram_tensor("bk", [D], F32, kind="ExternalInput")[:]
    bv_h = nc.dram_tensor("bv", [D], F32, kind="ExternalInput")[:]
    bo_h = nc.dram_tensor("bo", [D], F32, kind="ExternalInput")[:]
    gamma_h = nc.dram_tensor("gamma", [D], F32, kind="ExternalInput")[:]
    beta_h = nc.dram_tensor("beta", [D], F32, kind="ExternalInput")[:]
    out_h = nc.dram_tensor("out", [BLK, D], F32, kind="ExternalOutput")[:]
    attn_h = nc.dram_tensor("attn", [BLK, S], F32, kind="ExternalOutput")[:]

    with tile.TileContext(nc, pool_alloc_mode="queue") as tc, ExitStack() as top:
        singles = top.enter_context(tc.tile_pool(name="singles", bufs=1))
        ident = singles.tile([128, 128], F32)
        make_identity(nc, ident)
        c8e9 = singles.tile([128, min(512, S)], F32)
        nc.vector.memset(c8e9, MASK_FILL_RAW)
        bqc = singles.tile([128, KD], F32)
        nc.gpsimd.dma_start(out=bqc, in_=bq_h.rearrange("(t p) -> p t", p=128))
        bkc = singles.tile([128, KD], F32)
        nc.gpsimd.dma_start(out=bkc, in_=bk_h.rearrange("(t p) -> p t", p=128))
        bvb = singles.tile([128, D], F32)
        nc.gpsimd.dma_start(out=bvb, in_=_bcast(bv_h, 128))

        ctxres = top.enter_context(tc.tile_pool(name="ctxres", bufs=1))
        ctxr = ctxres.tile([128, NT, D], F32)  # (ctx + x) rows, tiled by query tile

        def make_transpose_chunk(bwork, bpsum):
            def transpose_chunk(src_rows, c0, cw):
                """PE-transpose x[c0:c0+cw, :] into an [128, KD, CH] x^T chunk."""
                xT = bwork.tile([128, KD, CH], F32, tag="xT", name="xT")
                for s0, sw in _split(cw, 128):
                    xr = bwork.tile([128, D], F32, tag="xr", name="xr")
                    nc.sync.dma_start(out=xr[:sw], in_=src_rows[c0 + s0:c0 + s0 + sw, :])
                    for g0 in range(0, KD, 4):
                        gn = min(4, KD - g0)
                        pst = bpsum.tile([128, 512], F32, tag="pst", name="pst")
                        for j in range(gn):
                            nc.tensor.transpose(
                                pst[:, j * 128:(j + 1) * 128],
                                xr[:, (g0 + j) * 128:(g0 + j + 1) * 128], ident)
                        nc.vector.tensor_copy(
                            out=xT[:, g0:g0 + gn, s0:s0 + sw],
                            in_=pst[:, :gn * 128].rearrange("p (g c) -> p g c", c=128))
                return xT
            return transpose_chunk

        # ------------- phase 1: q^T / k^T builds + scores/softmax -------------
        with ExitStack() as attn_scope:
            attres = attn_scope.enter_context(tc.tile_pool(name="attres", bufs=1))
            kT = attres.tile([128, KD, S], F32)     # k^T: [dk within tile, (dk tile, key row)]
            qT = attres.tile([128, KD, BLK], F32)   # q^T

            with ExitStack() as bctx:
                wpool = bctx.enter_context(tc.tile_pool(name="wpool", bufs=1))
                bwork = bctx.enter_context(tc.tile_pool(name="bwork", bufs=2))
                bpsum = bctx.enter_context(tc.tile_pool(name="bpsum", bufs=2, space="PSUM"))
                mpsum = bctx.enter_context(tc.tile_pool(name="mpsum", bufs=4, space="PSUM"))
                transpose_chunk = make_transpose_chunk(bwork, bpsum)

                # q^T build
                ws = wpool.tile([128, KD, D], F32, tag="w", name="wq_s")
                nc.sync.dma_start(out=ws, in_=wq_h.rearrange("(t p) n -> p t n", p=128))
                for c0, cw in _split(BLK, CH):
                    xT = transpose_chunk(xq_h, c0, cw)
                    for qd in range(KD):
                        for n0, nw in _split(cw, 512):
                            ps = mpsum.tile([128, 512], F32, tag="mm", name="mm")
                            for dt in range(KD):
                                nc.tensor.matmul(ps[:, :nw],
                                                 ws[:, dt, qd * 128:(qd + 1) * 128],
                                                 xT[:, dt, n0:n0 + nw],
                                                 start=dt == 0, stop=dt == KD - 1)
                            nc.vector.tensor_scalar_add(
                                out=qT[:, qd, c0 + n0:c0 + n0 + nw],
                                in0=ps[:, :nw], scalar1=bqc[:, qd:qd + 1])

                # k^T build
                ws = wpool.tile([128, KD, D], F32, tag="w", name="wk_s")
                nc.sync.dma_start(out=ws, in_=wk_h.rearrange("(t p) n -> p t n", p=128))
                for c0, cw in _split(S, CH):
                    xT = transpose_chunk(xf_h, c0, cw)
                    for kd in range(KD):
                        for n0, nw in _split(cw, 512):
                            ps = mpsum.tile([128, 512], F32, tag="mm", name="mm")
                            for dt in range(KD):
                                nc.tensor.matmul(ps[:, :nw],
                                                 ws[:, dt, kd * 128:(kd + 1) * 128],
                                                 xT[:, dt, n0:n0 + nw],
                                                 start=dt == 0, stop=dt == KD - 1)
                            nc.vector.tensor_scalar_add(
                                out=kT[:, kd, c0 + n0:c0 + n0 + nw],
                                in0=ps[:, :nw], scalar1=bkc[:, kd:kd + 1])

            # ---------------- attention: scores -> softmax -> attn out ----------------
            with ExitStack() as actx:
                apool = actx.enter_context(tc.tile_pool(name="apool", bufs=2))
                spsum = actx.enter_context(tc.tile_pool(name="spsum", bufs=8, space="PSUM"))
                nsc = len(_split(S, 512))
                for t in range(NT):
                    mt = apool.tile([128, S], U8, tag="mask")
                    nc.sync.dma_start(out=mt, in_=mask_h[t * 128:(t + 1) * 128, :])

                    # raw scores (q.k, unscaled), in 512-wide psum banks
                    pss = [spsum.tile([128, 512], F32, tag="ps_s", name=f"ps_s_{t}_{n}")
                           for n in range(nsc)]
                    for dt in range(KD):
                        for n, (n0, nw) in enumerate(_split(S, 512)):
                            nc.tensor.matmul(pss[n][:, :nw],
                                             qT[:, dt, t * 128:(t + 1) * 128],
                                             kT[:, dt, n0:n0 + nw],
                                             start=dt == 0, stop=dt == KD - 1)
                    # masked fill (raw domain)
                    for n, (n0, nw) in enumerate(_split(S, 512)):
                        nc.vector.copy_predicated(pss[n][:, :nw], mt[:, n0:n0 + nw],
                                                  c8e9[:, :nw])
                    # row max -> exp bias
                    mx4 = apool.tile([128, nsc], F32, tag="mx4")
                    for n, (n0, nw) in enumerate(_split(S, 512)):
                        nc.vector.reduce_max(mx4[:, n:n + 1], pss[n][:, :nw], axis=AX)
                    negb = apool.tile([128, 1], F32, tag="negb")
                    if nsc > 1:
                        mx = apool.tile([128, 1], F32, tag="mx")
                        nc.vector.reduce_max(mx, mx4, axis=AX)
                        nc.scalar.mul(out=negb, in_=mx, mul=-0.125)
                    else:
                        nc.scalar.mul(out=negb, in_=mx4, mul=-0.125)
                    # exp((s - max)/8) with per-chunk row sums
                    at = apool.tile([128, S], F32, tag="at")
                    sm = apool.tile([128, nsc], F32, tag="sm")
                    for n, (n0, nw) in enumerate(_split(S, 512)):
                        nc.scalar.activation(out=at[:, n0:n0 + nw], in_=pss[n][:, :nw],
                                             func=EXP, bias=negb, scale=0.125,
                                             accum_out=sm[:, n:n + 1])
                    z = apool.tile([128, 1], F32, tag="z")
                    nc.vector.reduce_sum(z, sm, axis=AX)
                    iz = apool.tile([128, 1], F32, tag="iz")
                    nc.vector.reciprocal(iz, z)
                    nc.vector.tensor_scalar_mul(at, at, iz)
                    nc.sync.dma_start(out=attn_h[t * 128:(t + 1) * 128, :], in_=at)

        # ------------- phase 2: V build + context (attn streamed back) -------------
        with ExitStack() as ctx_scope:
            vres = ctx_scope.enter_context(tc.tile_pool(name="vres", bufs=1))
            vt = vres.tile([128, NKC, D], F32)   # v: [key row within tile, (key tile, dv)]

            with ExitStack() as bctx:
                wpool = bctx.enter_context(tc.tile_pool(name="wpool2", bufs=1))
                bwork = bctx.enter_context(tc.tile_pool(name="bwork2", bufs=2))
                bpsum = bctx.enter_context(tc.tile_pool(name="bpsum2", bufs=2, space="PSUM"))
                mpsum = bctx.enter_context(tc.tile_pool(name="mpsum2", bufs=4, space="PSUM"))
                transpose_chunk = make_transpose_chunk(bwork, bpsum)

                ws = wpool.tile([128, KD, D], F32, tag="w", name="wv_s")
                nc.sync.dma_start(out=ws, in_=wv_h.rearrange("(t p) n -> p t n", p=128))
                for c0, cw in _split(S, CH):
                    xT = transpose_chunk(xf_h, c0, cw)
                    for r0 in range(0, cw, 128):
                        kc = (c0 + r0) // 128
                        for h0, hw in _split(D, 512):
                            ps = mpsum.tile([128, 512], F32, tag="mm", name="mm")
                            for dt in range(KD):
                                nc.tensor.matmul(ps[:, :hw],
                                                 xT[:, dt, r0:r0 + 128],
                                                 ws[:, dt, h0:h0 + hw],
                                                 start=dt == 0, stop=dt == KD - 1)
                            nc.vector.tensor_tensor(
                                out=vt[:, kc, h0:h0 + hw], in0=ps[:, :hw],
                                in1=bvb[:, h0:h0 + hw], op=ADD)

            with ExitStack() as cctx:
                cpool = cctx.enter_context(tc.tile_pool(name="cpool", bufs=2))
                tpsum = cctx.enter_context(tc.tile_pool(name="tpsum", bufs=4, space="PSUM"))
                cpsum = cctx.enter_context(tc.tile_pool(name="cpsum", bufs=4, space="PSUM"))
                for t in range(NT):
                    ar = cpool.tile([128, S], F32, tag="ar")
                    nc.sync.dma_start(out=ar, in_=attn_h[t * 128:(t + 1) * 128, :])
                    xqr = cpool.tile([128, D], F32, tag="xqr")
                    nc.sync.dma_start(out=xqr, in_=xq_h[t * 128:(t + 1) * 128, :])
                    aT = cpool.tile([128, NKC, 128], F32, tag="aT")
                    for g0 in range(0, NKC, 4):
                        gn = min(4, NKC - g0)
                        pst = tpsum.tile([128, 512], F32, tag="pst", name="pst")
                        for j in range(gn):
                            nc.tensor.transpose(pst[:, j * 128:(j + 1) * 128],
                                                ar[:, (g0 + j) * 128:(g0 + j + 1) * 128],
                                                ident)
                        nc.vector.tensor_copy(
                            out=aT[:, g0:g0 + gn, :],
                            in_=pst[:, :gn * 128].rearrange("p (g c) -> p g c", c=128))
                    # context + residual
                    for h, (h0, hw) in enumerate(_split(D, 512)):
                        psc = cpsum.tile([128, 512], F32, tag="psc", name="psc")
                        for c in range(NKC):
                            nc.tensor.matmul(psc[:, :hw], aT[:, c, :],
                                             vt[:, c, h0:h0 + hw],
                                             start=c == 0, stop=c == NKC - 1)
                        nc.vector.tensor_tensor(out=ctxr[:, t, h0:h0 + hw],
                                                in0=psc[:, :hw],
                                                in1=xqr[:, h0:h0 + hw], op=ADD)

        # ---------------- output projection + layernorm ----------------
        with ExitStack() as octx:
            osing = octx.enter_context(tc.tile_pool(name="osing", bufs=1))
            opool = octx.enter_context(tc.tile_pool(name="opool", bufs=2))
            opsum = octx.enter_context(tc.tile_pool(name="opsum", bufs=2, space="PSUM"))
            ot_psum = octx.enter_context(tc.tile_pool(name="otpsum", bufs=2, space="PSUM"))
            wos = osing.tile([128, KD, D], F32)
            nc.sync.dma_start(out=wos, in_=wo_h.rearrange("(t p) n -> p t n", p=128))
            bob = osing.tile([128, D], F32)
            nc.gpsimd.dma_start(out=bob, in_=_bcast(bo_h, 128))
            gb = osing.tile([128, D], F32)
            nc.gpsimd.dma_start(out=gb, in_=_bcast(gamma_h, 128))
            bb = osing.tile([128, D], F32)
            nc.gpsimd.dma_start(out=bb, in_=_bcast(beta_h, 128))
            epst = osing.tile([128, 1], F32)
            nc.vector.memset(epst, LN_EPS)

            nln = len(_split(D, 512))
            for t in range(NT):
                cT = opool.tile([128, KD, 128], F32, tag="cT")
                for g0 in range(0, KD, 4):
                    gn = min(4, KD - g0)
                    pst = ot_psum.tile([128, 512], F32, tag="pst")
                    for j in range(gn):
                        nc.tensor.transpose(pst[:, j * 128:(j + 1) * 128],
                                            ctxr[:, t, (g0 + j) * 128:(g0 + j + 1) * 128],
                                            ident)
                    nc.vector.tensor_copy(
                        out=cT[:, g0:g0 + gn, :],
                        in_=pst[:, :gn * 128].rearrange("p (g c) -> p g c", c=128))
                y = opool.tile([128, D], F32, tag="y")
                for h0, hw in _split(D, 512):
                    psy = opsum.tile([128, 512], F32, tag="psy")
                    for dt in range(KD):
                        nc.tensor.matmul(psy[:, :hw], cT[:, dt, :],
                                         wos[:, dt, h0:h0 + hw],
                                         start=dt == 0, stop=dt == KD - 1)
                    nc.vector.tensor_tensor(out=y[:, h0:h0 + hw], in0=psy[:, :hw],
                                            in1=bob[:, h0:h0 + hw], op=ADD)
                # layernorm
                stats = opool.tile([128, nln, 6], F32, tag="st")
                for g, (g0, gw) in enumerate(_split(D, 512)):
                    nc.vector.bn_stats(out=stats[:, g, :], in_=y[:, g0:g0 + gw])
                mv = opool.tile([128, 2], F32, tag="mv")
                nc.vector.bn_aggr(out=mv, in_=stats)
                rst = opool.tile([128, 1], F32, tag="rst")
                nc.scalar.activation(out=rst, in_=mv[:, 1:2], func=SQRT,
                                     bias=epst, scale=1.0)
                nc.vector.reciprocal(rst, rst)
                y2 = opool.tile([128, D], F32, tag="y2")
                nc.vector.tensor_scalar(out=y2, in0=y, scalar1=mv[:, 0:1],
                                        scalar2=rst, op0=SUB, op1=MULT)
                nc.vector.tensor_mul(y2, y2, gb)
                nc.vector.tensor_add(y2, y2, bb)
                nc.sync.dma_start(out=out_h[t * 128:(t + 1) * 128, :], in_=y2)

    return nc


_NC_CACHE = {}


def _run(inputs, trace=False, trace_kwargs=None):
    x = np.asarray(inputs["x"], dtype=np.float32)
    mask = np.asarray(inputs["attn_mask"])
    ws = {k: np.ascontiguousarray(np.asarray(inputs[k], dtype=np.float32))
          for k in ("wq", "wk", "wv", "wo", "bq", "bk", "bv", "bo", "gamma", "beta")}
    B, S, D = x.shape
    BLK = S // 2

    from concourse.bass_utils import run_bass_kernel_spmd
    key = (S, D, BLK)
    if key not in _NC_CACHE:
        nc = build_mha(S=S, D=D, BLK=BLK)
        split_waits(nc)
        _NC_CACHE[key] = nc
    nc = _NC_CACHE[key]

    in_maps = []
    for core in range(8):
        b, h = core // 2, core % 2
        blk = slice(h * BLK, (h + 1) * BLK)
        m = {"xf": np.ascontiguousarray(x[b]),
             "xq": np.ascontiguousarray(x[b, blk]),
             "mask": np.ascontiguousarray(mask[b, blk]).astype(np.uint8)}
        m.update(ws)
        in_maps.append(m)

    res = run_bass_kernel_spmd(nc, in_maps, core_ids=list(range(8)),
                               trace=trace, **(trace_kwargs or {}))

    out = np.empty((B, S, D), np.float32)
    attn = np.empty((B, S, S), np.float32)
    for core in range(8):
        b, h = core // 2, core % 2
        blk = slice(h * BLK, (h + 1) * BLK)
        out[b, blk] = res.results[core]["out"]
        attn[b, blk] = res.results[core]["attn"]
    return out, attn, res


def kernel(**inputs):
    out, attn, _ = _run(inputs)
    return out, attn
